# revision 2
# baseline (speedup 1.0000x reference)
"""Sharded top-1 KNN (retrieval) on 8 TRN2 NeuronCores via Bass/Tile.

Strategy (hardcoded for x[2048,24,16], X_train[65536,384], Y_train[65536,24,1]):
  - Shard X_train rows across 8 cores (8192 rows each).
  - Host pre-transposes x -> xT [384,2048] bf16 and each X_train shard ->
    XT [384,8192] bf16, and precomputes ttb = broadcast(||t||^2/2) [128,8192] bf16.
  - Each core computes scores s[q,n] = (x.t)_bf16 - ||t||^2/2 tile-by-tile
    (TensorE bf16 matmuls into PSUM, VectorE subtract-drain into an SBUF row
    buffer), then per query takes top-8 values + column indices (VectorE
    max/max_index).  argmin(d^2) == argmax(s), and the true nearest neighbor
    is in the bf16 top-8 with overwhelming probability (score noise sigma
    ~1 absolute vs. expected min-gap ~4 in d^2 units; a miss needs 8
    independent ~5..30-sigma events).
  - Host gathers 8 cores x top-8 = 64 candidates per query, recomputes exact
    distances in float64 for just those candidates, picks the argmin (ties:
    smallest global index, matching jnp.argmin), and returns Y_train[best].
"""

import os
import sys

import numpy as np

for _p in ("/opt/trn_rl_repo",):
    if os.path.isdir(_p) and _p not in sys.path:
        sys.path.insert(0, _p)

import ml_dtypes  # noqa: E402

B, T, F = 2048, 24, 16
D = T * F  # 384
N = 65536
NCORES = 8
NS = N // NCORES  # 8192 rows per core
KT = D // 128  # 3 k-tiles
MT = B // 128  # 16 query tiles
NCHUNK = 512
NT = NS // NCHUNK  # 16 train chunks per core
NGROUP = 8  # psum tiles in flight per group
TOPK = 8

_BF16 = ml_dtypes.bfloat16


def build_nc(b=B, ns=NS, d=D):
    """Build the per-core Bass program (SPMD: same program, per-core inputs)."""
    import concourse.tile as tile
    from concourse import bacc, mybir

    kt = d // 128
    mt = b // 128
    nt = ns // NCHUNK

    nc = bacc.Bacc(None, target_bir_lowering=False)
    xT = nc.dram_tensor("xT", [d, b], mybir.dt.bfloat16, kind="ExternalInput")
    XT = nc.dram_tensor("XT", [d, ns], mybir.dt.bfloat16, kind="ExternalInput")
    ttb = nc.dram_tensor("ttb", [128, ns], mybir.dt.bfloat16, kind="ExternalInput")
    idx_out = nc.dram_tensor("idx8", [b, TOPK], mybir.dt.uint32, kind="ExternalOutput")

    with tile.TileContext(nc) as tc:
        with (
            tc.tile_pool(name="wpool", bufs=1) as wpool,
            tc.tile_pool(name="rpool", bufs=2) as rpool,
            tc.tile_pool(name="ppool", bufs=NGROUP, space="PSUM") as ppool,
            tc.tile_pool(name="spool", bufs=4) as spool,
        ):
            xT_sb = []
            XT_sb = []
            for k in range(kt):
                xk = wpool.tile([128, b], mybir.dt.bfloat16, name="xk", tag=f"xk{k}")
                nc.sync.dma_start(xk[:], xT[k * 128 : (k + 1) * 128, :])
                xT_sb.append(xk)
                tk = wpool.tile([128, ns], mybir.dt.bfloat16, name="tk", tag=f"tk{k}")
                nc.sync.dma_start(tk[:], XT[k * 128 : (k + 1) * 128, :])
                XT_sb.append(tk)
            tt_sb = wpool.tile([128, ns], mybir.dt.bfloat16, name="tt_sb", tag="tt_sb")
            nc.sync.dma_start(tt_sb[:], ttb[:, :])

            for m in range(mt):
                rowbuf = rpool.tile([128, ns], mybir.dt.bfloat16, name="rowbuf")
                for g in range(0, nt, NGROUP):
                    gn = min(NGROUP, nt - g)
                    pss = [
                        ppool.tile([128, NCHUNK], mybir.dt.float32, name="ps", tag="ps")
                        for _ in range(gn)
                    ]
                    # k outer, n inner: the stationary operand (xT m-tile)
                    # stays resident across the inner loop.
                    for k in range(kt):
                        for j in range(gn):
                            n = g + j
                            nc.tensor.matmul(
                                pss[j][:],
                                xT_sb[k][:, m * 128 : (m + 1) * 128],
                                XT_sb[k][:, n * NCHUNK : (n + 1) * NCHUNK],
                                start=(k == 0),
                                stop=(k == kt - 1),
                            )
                    for j in range(gn):
                        n = g + j
                        nc.vector.tensor_sub(
                            rowbuf[:, n * NCHUNK : (n + 1) * NCHUNK],
                            pss[j][:],
                            tt_sb[:, n * NCHUNK : (n + 1) * NCHUNK],
                        )
                max8 = spool.tile([128, TOPK], mybir.dt.bfloat16, name="max8")
                idx8 = spool.tile([128, TOPK], mybir.dt.uint32, name="idx8t")
                nc.vector.max(max8[:], rowbuf[:])
                nc.vector.max_index(idx8[:], max8[:], rowbuf[:])
                nc.sync.dma_start(idx_out[m * 128 : (m + 1) * 128, :], idx8[:])
    nc.finalize()  # Bacc register allocation; walrus rejects unfinalized BIR
    return nc


_NC = None


def _get_nc():
    global _NC
    if _NC is None:
        _NC = build_nc()
    return _NC


def _prep_in_maps(xf, X_train):
    xT_b = np.ascontiguousarray(xf.T).astype(_BF16)
    in_maps = []
    for c in range(NCORES):
        Xs = X_train[c * NS : (c + 1) * NS]
        XT_b = np.ascontiguousarray(Xs.T).astype(_BF16)
        tt_half = (Xs.astype(np.float64) ** 2).sum(axis=1) * 0.5
        ttb_b = np.ascontiguousarray(
            np.broadcast_to(tt_half.astype(np.float32)[None, :], (128, NS))
        ).astype(_BF16)
        in_maps.append({"xT": xT_b, "XT": XT_b, "ttb": ttb_b})
    return in_maps


def _refine(xf, X_train, Y_train, idx_all):
    """idx_all: [B, NCORES*TOPK] global candidate indices (int64)."""
    cand = np.sort(idx_all, axis=1)
    Xc = X_train[cand].astype(np.float64)  # [B, C, D]
    diff = xf.astype(np.float64)[:, None, :] - Xc
    d2 = np.einsum("bcd,bcd->bc", diff, diff)
    best = cand[np.arange(cand.shape[0]), np.argmin(d2, axis=1)]
    return Y_train[best].astype(np.float32)


def kernel(x, X_train, Y_train, _trace=False, _tmpdir=None):
    from concourse.bass_utils import run_bass_kernel_spmd

    x = np.asarray(x, dtype=np.float32)
    X_train = np.asarray(X_train, dtype=np.float32)
    Y_train = np.asarray(Y_train, dtype=np.float32)
    xf = x.reshape(B, D)

    in_maps = _prep_in_maps(xf, X_train)
    nc = _get_nc()
    kw = {}
    if _trace:
        kw = {"trace": True, "tmpdir": _tmpdir}
    res = run_bass_kernel_spmd(nc, in_maps, core_ids=list(range(NCORES)), **kw)

    idx_all = np.concatenate(
        [
            np.minimum(res.results[c]["idx8"].astype(np.int64), NS - 1) + c * NS
            for c in range(NCORES)
        ],
        axis=1,
    )  # [B, 64]
    out = _refine(xf, X_train, Y_train, idx_all)
    if _trace:
        return out, res
    return out


# revision 3
# speedup vs baseline: 1.7274x; 1.7274x over previous
"""Sharded top-1 KNN (retrieval) on 8 TRN2 NeuronCores via Bass/Tile.

v2 strategy (hardcoded for x[2048,24,16], X_train[65536,384], Y_train[65536,24,1]):
  - Shard X_train rows across 8 cores (8192 rows each).
  - Host pre-transposes x -> xT [384,2048] bf16 and each (permuted) X_train
    shard -> XT [384,8192] bf16.  The shard rows are permuted so that the 4
    rows any folded score-column mixes (see below) are adjacent in the
    ||t||^2 order, letting one shared bias serve all 4.
  - Each core computes cross = x.t (bf16 TensorE matmuls into PSUM).  The 16
    psum chunks of a query-tile row are max-FOLDED 4->1 during the drain
    (VectorE copy/max), giving a [128,2048] pooled score row.  A single
    bf16 subtract applies the shared -||t||^2/2 bias post-fold, then
    MAX8/FIND_INDEX8 produce top-8 pooled positions per query.
  - Each pooled position covers 4 training rows; the host expands 8 cores x
    top-8 x 4 = 256 candidates per query, recomputes exact distances in
    float64 for just those, picks the argmin (ties: smallest global index,
    matching jnp.argmin), and returns Y_train[best].
  Max-pooling cannot hurt candidate recall: the true NN's pooled column
  value >= its own score, and every competing pooled column is the max of
  rows that individually rank below it, so pooled-rank(true NN) <=
  raw-rank(true NN) (empirically <= 2 on this data, vs the 8 kept).
"""

import os
import sys

import numpy as np

for _p in ("/opt/trn_rl_repo",):
    if os.path.isdir(_p) and _p not in sys.path:
        sys.path.insert(0, _p)

import ml_dtypes  # noqa: E402

B, T, F = 2048, 24, 16
D = T * F  # 384
N = 65536
NCORES = 8
NS = N // NCORES  # 8192 rows per core
KT = D // 128  # 3 k-tiles
MT = B // 128  # 16 query tiles
NCHUNK = 512
NT = NS // NCHUNK  # 16 train chunks per core
NGROUP = 8  # psum tiles in flight per group
FOLD = 4  # chunks max-folded into one scan column
NFOLD = NS // FOLD  # 2048 pooled positions
TOPK = 8

_BF16 = ml_dtypes.bfloat16


def build_nc(b=B, ns=NS, d=D):
    """Build the per-core Bass program (SPMD: same program, per-core inputs)."""
    import concourse.tile as tile
    from concourse import bacc, mybir

    kt = d // 128
    mt = b // 128
    nt = ns // NCHUNK
    nfold = ns // FOLD

    nc = bacc.Bacc(None, target_bir_lowering=False)
    xT = nc.dram_tensor("xT", [d, b], mybir.dt.bfloat16, kind="ExternalInput")
    XT = nc.dram_tensor("XT", [d, ns], mybir.dt.bfloat16, kind="ExternalInput")
    ttf = nc.dram_tensor("ttf", [128, nfold], mybir.dt.bfloat16, kind="ExternalInput")
    idx_out = nc.dram_tensor("idx8", [b, TOPK], mybir.dt.uint32, kind="ExternalOutput")

    with tile.TileContext(nc) as tc:
        with (
            tc.tile_pool(name="wpool", bufs=1) as wpool,
            tc.tile_pool(name="rpool", bufs=2) as rpool,
            tc.tile_pool(name="ppool", bufs=NGROUP, space="PSUM") as ppool,
            tc.tile_pool(name="spool", bufs=4) as spool,
        ):
            xT_sb = []
            XT_sb = []
            for k in range(kt):
                xk = wpool.tile([128, b], mybir.dt.bfloat16, name="xk", tag=f"xk{k}")
                nc.sync.dma_start(xk[:], xT[k * 128 : (k + 1) * 128, :])
                xT_sb.append(xk)
                tk = wpool.tile([128, ns], mybir.dt.bfloat16, name="tk", tag=f"tk{k}")
                nc.sync.dma_start(tk[:], XT[k * 128 : (k + 1) * 128, :])
                XT_sb.append(tk)
            tt_sb = wpool.tile([128, nfold], mybir.dt.bfloat16, name="tt_sb", tag="tt")
            nc.sync.dma_start(tt_sb[:], ttf[:, :])

            for m in range(mt):
                vmax = rpool.tile([128, nfold], mybir.dt.bfloat16, name="vmax")
                for g in range(0, nt, NGROUP):
                    gn = min(NGROUP, nt - g)
                    pss = [
                        ppool.tile([128, NCHUNK], mybir.dt.float32, name="ps", tag="ps")
                        for _ in range(gn)
                    ]
                    # k outer, n inner: the stationary operand (xT m-tile)
                    # stays resident across the inner loop.
                    for k in range(kt):
                        for j in range(gn):
                            n = g + j
                            nc.tensor.matmul(
                                pss[j][:],
                                xT_sb[k][:, m * 128 : (m + 1) * 128],
                                XT_sb[k][:, n * NCHUNK : (n + 1) * NCHUNK],
                                start=(k == 0),
                                stop=(k == kt - 1),
                            )
                    # drain with 4->1 max-fold: chunk n lands in pooled
                    # slice n//FOLD; emit the two fold-chains interleaved
                    # (i-outer) so VectorE never waits on its own chain.
                    for i in range(FOLD):
                        for fg in range(gn // FOLD):
                            j = fg * FOLD + i
                            n = g + j
                            dstslice = vmax[
                                :,
                                (n // FOLD) * NCHUNK : (n // FOLD + 1) * NCHUNK,
                            ]
                            if i == 0:
                                nc.vector.tensor_copy(dstslice, pss[j][:])
                            else:
                                nc.vector.tensor_tensor(
                                    dstslice,
                                    pss[j][:],
                                    dstslice,
                                    op=mybir.AluOpType.max,
                                )
                # shared bias post-fold (all-bf16 SBUF -> DVE 2x mode)
                nc.vector.tensor_sub(vmax[:], vmax[:], tt_sb[:])
                max8 = spool.tile([128, TOPK], mybir.dt.bfloat16, name="max8")
                idx8 = spool.tile([128, TOPK], mybir.dt.uint32, name="idx8t")
                nc.vector.max(max8[:], vmax[:])
                nc.vector.max_index(idx8[:], max8[:], vmax[:])
                nc.sync.dma_start(idx_out[m * 128 : (m + 1) * 128, :], idx8[:])
    nc.finalize()  # Bacc register allocation; walrus rejects unfinalized BIR
    return nc


_NC = None


def _get_nc():
    global _NC
    if _NC is None:
        _NC = build_nc()
    return _NC


def _shard_perm(tt, ns):
    """Permutation placing tt-sorted rows so each folded quad is tt-adjacent.

    Device row n = (FOLD*g + i)*NCHUNK + col (g = fold group, col = scan
    column) folds with i = 0..FOLD-1.  Give it sorted rank
    (g*NCHUNK + col)*FOLD + i so the 4 folded rows are consecutive in tt.
    """
    order = np.argsort(tt, kind="stable")  # sorted rank -> original row
    n = np.arange(ns)
    chunk = n // NCHUNK
    col = n % NCHUNK
    g = chunk // FOLD
    i = chunk % FOLD
    rank = (g * NCHUNK + col) * FOLD + i
    return order[rank]  # device row n holds original row perm[n]


def _prep_in_maps(xf, X_train):
    xT_b = np.ascontiguousarray(xf.T).astype(_BF16)
    in_maps = []
    perms = []
    for c in range(NCORES):
        Xs = X_train[c * NS : (c + 1) * NS]
        tt = (Xs.astype(np.float64) ** 2).sum(axis=1)
        perm = _shard_perm(tt, NS)
        perms.append(perm)
        XT_b = np.ascontiguousarray(Xs[perm].T).astype(_BF16)
        # shared bias per pooled position = mean tt/2 of its folded quad
        tt_dev = tt[perm] * 0.5  # tt of device row n
        quad = tt_dev.reshape(NT // FOLD, FOLD, NCHUNK)  # [g, i, col]
        ttf = quad.mean(axis=1).reshape(NFOLD)  # [g*NCHUNK + col]
        ttf_b = np.ascontiguousarray(
            np.broadcast_to(ttf.astype(np.float32)[None, :], (128, NFOLD))
        ).astype(_BF16)
        in_maps.append({"xT": xT_b, "XT": XT_b, "ttf": ttf_b})
    return in_maps, perms


def _refine(xf, X_train, Y_train, cand):
    """cand: [B, C] global candidate row indices (int64, may repeat)."""
    b = cand.shape[0]
    cand = np.sort(cand, axis=1)
    best = np.empty(b, dtype=np.int64)
    xd = xf.astype(np.float64)
    step = 128
    for s in range(0, b, step):
        e = min(s + step, b)
        Xc = X_train[cand[s:e]].astype(np.float64)  # [q, C, D]
        diff = xd[s:e, None, :] - Xc
        d2 = np.einsum("qcd,qcd->qc", diff, diff)
        best[s:e] = cand[s:e][np.arange(e - s), np.argmin(d2, axis=1)]
    return Y_train[best].astype(np.float32)


def kernel(x, X_train, Y_train, _trace=False, _tmpdir=None):
    from concourse.bass_utils import run_bass_kernel_spmd

    x = np.asarray(x, dtype=np.float32)
    X_train = np.asarray(X_train, dtype=np.float32)
    Y_train = np.asarray(Y_train, dtype=np.float32)
    xf = x.reshape(B, D)

    in_maps, perms = _prep_in_maps(xf, X_train)
    nc = _get_nc()
    kw = {}
    if _trace:
        kw = {"trace": True, "tmpdir": _tmpdir}
    res = run_bass_kernel_spmd(nc, in_maps, core_ids=list(range(NCORES)), **kw)

    # pooled position p -> device rows (FOLD*(p//NCHUNK) + i)*NCHUNK + p%NCHUNK
    cands = []
    for c in range(NCORES):
        p = np.minimum(res.results[c]["idx8"].astype(np.int64), NFOLD - 1)  # [B,8]
        g, col = p // NCHUNK, p % NCHUNK
        devrows = (
            (FOLD * g[:, :, None] + np.arange(FOLD)[None, None, :]) * NCHUNK
            + col[:, :, None]
        ).reshape(B, TOPK * FOLD)
        cands.append(perms[c][devrows] + c * NS)
    cand = np.concatenate(cands, axis=1)  # [B, 256]
    out = _refine(xf, X_train, Y_train, cand)
    if _trace:
        return out, res
    return out


# revision 4
# speedup vs baseline: 2.0927x; 1.2115x over previous
"""Sharded top-1 KNN (retrieval) on 8 TRN2 NeuronCores via Bass/Tile.

v2 strategy (hardcoded for x[2048,24,16], X_train[65536,384], Y_train[65536,24,1]):
  - Shard X_train rows across 8 cores (8192 rows each).
  - Host pre-transposes x -> xT [384,2048] bf16 and each (permuted) X_train
    shard -> XT [384,8192] bf16.  The shard rows are permuted so that the 4
    rows any folded score-column mixes (see below) are adjacent in the
    ||t||^2 order, letting one shared bias serve all 4.
  - Each core computes cross = x.t (bf16 TensorE matmuls into PSUM).  The 16
    psum chunks of a query-tile row are max-FOLDED 4->1 during the drain
    (VectorE copy/max), giving a [128,2048] pooled score row.  A single
    bf16 subtract applies the shared -||t||^2/2 bias post-fold, then
    MAX8/FIND_INDEX8 produce top-8 pooled positions per query.
  - Each pooled position covers 4 training rows; the host expands 8 cores x
    top-8 x 4 = 256 candidates per query, recomputes exact distances in
    float64 for just those, picks the argmin (ties: smallest global index,
    matching jnp.argmin), and returns Y_train[best].
  Max-pooling cannot hurt candidate recall: the true NN's pooled column
  value >= its own score, and every competing pooled column is the max of
  rows that individually rank below it, so pooled-rank(true NN) <=
  raw-rank(true NN) (empirically <= 2 on this data, vs the 8 kept).
"""

import os
import sys

import numpy as np

for _p in ("/opt/trn_rl_repo",):
    if os.path.isdir(_p) and _p not in sys.path:
        sys.path.insert(0, _p)

import ml_dtypes  # noqa: E402

B, T, F = 2048, 24, 16
D = T * F  # 384
N = 65536
NCORES = 8
NS = N // NCORES  # 8192 rows per core
KT = D // 128  # 3 k-tiles
MT = B // 128  # 16 query tiles
NCHUNK = 512
NT = NS // NCHUNK  # 16 train chunks per core
NGROUP = 8  # psum tiles in flight per group
FOLD = 4  # chunks max-folded into one scan column
NFOLD = NS // FOLD  # 2048 pooled positions
TOPK = 8

_BF16 = ml_dtypes.bfloat16


def build_nc(b=B, ns=NS, d=D):
    """Build the per-core Bass program (SPMD: same program, per-core inputs)."""
    import concourse.tile as tile
    from concourse import bacc, mybir

    kt = d // 128
    mt = b // 128
    nt = ns // NCHUNK
    nfold = ns // FOLD

    nc = bacc.Bacc(None, target_bir_lowering=False)
    xT = nc.dram_tensor("xT", [d, b], mybir.dt.bfloat16, kind="ExternalInput")
    XT = nc.dram_tensor("XT", [d, ns], mybir.dt.bfloat16, kind="ExternalInput")
    ttf = nc.dram_tensor("ttf", [128, nfold], mybir.dt.bfloat16, kind="ExternalInput")
    idx_out = nc.dram_tensor("idx8", [b, TOPK], mybir.dt.uint32, kind="ExternalOutput")

    with tile.TileContext(nc) as tc:
        with (
            tc.tile_pool(name="wpool", bufs=1) as wpool,
            tc.tile_pool(name="rpool", bufs=2) as rpool,
            tc.tile_pool(name="ppool", bufs=NGROUP, space="PSUM") as ppool,
            tc.tile_pool(name="spool", bufs=4) as spool,
        ):
            xT_sb = []
            XT_sb = []
            for k in range(kt):
                xk = wpool.tile([128, b], mybir.dt.bfloat16, name="xk", tag=f"xk{k}")
                nc.sync.dma_start(xk[:], xT[k * 128 : (k + 1) * 128, :])
                xT_sb.append(xk)
                tk = wpool.tile([128, ns], mybir.dt.bfloat16, name="tk", tag=f"tk{k}")
                nc.sync.dma_start(tk[:], XT[k * 128 : (k + 1) * 128, :])
                XT_sb.append(tk)
            tt_sb = wpool.tile([128, nfold], mybir.dt.bfloat16, name="tt_sb", tag="tt")
            nc.sync.dma_start(tt_sb[:], ttf[:, :])

            for m in range(mt):
                vmax = rpool.tile([128, nfold], mybir.dt.bfloat16, name="vmax")
                for g in range(0, nt, NGROUP):
                    gn = min(NGROUP, nt - g)
                    pss = [
                        ppool.tile([128, NCHUNK], mybir.dt.float32, name="ps", tag="ps")
                        for _ in range(gn)
                    ]
                    # k outer, n inner: the stationary operand (xT m-tile)
                    # stays resident across the inner loop.
                    for k in range(kt):
                        for j in range(gn):
                            n = g + j
                            nc.tensor.matmul(
                                pss[j][:],
                                xT_sb[k][:, m * 128 : (m + 1) * 128],
                                XT_sb[k][:, n * NCHUNK : (n + 1) * NCHUNK],
                                start=(k == 0),
                                stop=(k == kt - 1),
                            )
                    # drain with 4->1 max-fold.  ScalarE (idle otherwise)
                    # casts chunks 0 and 2 of each quad out of PSUM; VectorE
                    # max-folds chunks 1 and 3 against them (one PSUM read
                    # each) and merges the two halves in fast all-bf16 mode.
                    for fg in range(gn // FOLD):
                        j = fg * FOLD
                        n = g + j
                        dstslice = vmax[
                            :, (n // FOLD) * NCHUNK : (n // FOLD + 1) * NCHUNK
                        ]
                        t0 = spool.tile(
                            [128, NCHUNK], mybir.dt.bfloat16, name="t0", tag="t0"
                        )
                        t1 = spool.tile(
                            [128, NCHUNK], mybir.dt.bfloat16, name="t1", tag="t1"
                        )
                        nc.scalar.copy(t0[:], pss[j][:])
                        nc.scalar.copy(t1[:], pss[j + 2][:])
                        nc.vector.tensor_tensor(
                            dstslice, pss[j + 1][:], t0[:], op=mybir.AluOpType.max
                        )
                        nc.vector.tensor_tensor(
                            t1[:], pss[j + 3][:], t1[:], op=mybir.AluOpType.max
                        )
                        nc.vector.tensor_tensor(
                            dstslice, dstslice, t1[:], op=mybir.AluOpType.max
                        )
                # shared bias post-fold (all-bf16 SBUF -> DVE 2x mode)
                nc.vector.tensor_sub(vmax[:], vmax[:], tt_sb[:])
                max8 = spool.tile([128, TOPK], mybir.dt.bfloat16, name="max8")
                idx8 = spool.tile([128, TOPK], mybir.dt.uint32, name="idx8t")
                nc.vector.max(max8[:], vmax[:])
                nc.vector.max_index(idx8[:], max8[:], vmax[:])
                nc.sync.dma_start(idx_out[m * 128 : (m + 1) * 128, :], idx8[:])
    nc.finalize()  # Bacc register allocation; walrus rejects unfinalized BIR
    return nc


_NC = None


def _get_nc():
    global _NC
    if _NC is None:
        _NC = build_nc()
    return _NC


def _shard_perm(tt, ns):
    """Permutation placing tt-sorted rows so each folded quad is tt-adjacent.

    Device row n = (FOLD*g + i)*NCHUNK + col (g = fold group, col = scan
    column) folds with i = 0..FOLD-1.  Give it sorted rank
    (g*NCHUNK + col)*FOLD + i so the 4 folded rows are consecutive in tt.
    """
    order = np.argsort(tt, kind="stable")  # sorted rank -> original row
    n = np.arange(ns)
    chunk = n // NCHUNK
    col = n % NCHUNK
    g = chunk // FOLD
    i = chunk % FOLD
    rank = (g * NCHUNK + col) * FOLD + i
    return order[rank]  # device row n holds original row perm[n]


def _prep_in_maps(xf, X_train):
    xT_b = np.ascontiguousarray(xf.T).astype(_BF16)
    in_maps = []
    perms = []
    for c in range(NCORES):
        Xs = X_train[c * NS : (c + 1) * NS]
        tt = (Xs.astype(np.float64) ** 2).sum(axis=1)
        perm = _shard_perm(tt, NS)
        perms.append(perm)
        XT_b = np.ascontiguousarray(Xs[perm].T).astype(_BF16)
        # shared bias per pooled position = mean tt/2 of its folded quad
        tt_dev = tt[perm] * 0.5  # tt of device row n
        quad = tt_dev.reshape(NT // FOLD, FOLD, NCHUNK)  # [g, i, col]
        ttf = quad.mean(axis=1).reshape(NFOLD)  # [g*NCHUNK + col]
        ttf_b = np.ascontiguousarray(
            np.broadcast_to(ttf.astype(np.float32)[None, :], (128, NFOLD))
        ).astype(_BF16)
        in_maps.append({"xT": xT_b, "XT": XT_b, "ttf": ttf_b})
    return in_maps, perms


def _refine(xf, X_train, Y_train, cand):
    """cand: [B, C] global candidate row indices (int64, may repeat)."""
    b = cand.shape[0]
    cand = np.sort(cand, axis=1)
    best = np.empty(b, dtype=np.int64)
    xd = xf.astype(np.float64)
    step = 128
    for s in range(0, b, step):
        e = min(s + step, b)
        Xc = X_train[cand[s:e]].astype(np.float64)  # [q, C, D]
        diff = xd[s:e, None, :] - Xc
        d2 = np.einsum("qcd,qcd->qc", diff, diff)
        best[s:e] = cand[s:e][np.arange(e - s), np.argmin(d2, axis=1)]
    return Y_train[best].astype(np.float32)


def kernel(x, X_train, Y_train, _trace=False, _tmpdir=None):
    from concourse.bass_utils import run_bass_kernel_spmd

    x = np.asarray(x, dtype=np.float32)
    X_train = np.asarray(X_train, dtype=np.float32)
    Y_train = np.asarray(Y_train, dtype=np.float32)
    xf = x.reshape(B, D)

    in_maps, perms = _prep_in_maps(xf, X_train)
    nc = _get_nc()
    kw = {}
    if _trace:
        kw = {"trace": True, "tmpdir": _tmpdir}
    res = run_bass_kernel_spmd(nc, in_maps, core_ids=list(range(NCORES)), **kw)

    # pooled position p -> device rows (FOLD*(p//NCHUNK) + i)*NCHUNK + p%NCHUNK
    cands = []
    for c in range(NCORES):
        p = np.minimum(res.results[c]["idx8"].astype(np.int64), NFOLD - 1)  # [B,8]
        g, col = p // NCHUNK, p % NCHUNK
        devrows = (
            (FOLD * g[:, :, None] + np.arange(FOLD)[None, None, :]) * NCHUNK
            + col[:, :, None]
        ).reshape(B, TOPK * FOLD)
        cands.append(perms[c][devrows] + c * NS)
    cand = np.concatenate(cands, axis=1)  # [B, 256]
    out = _refine(xf, X_train, Y_train, cand)
    if _trace:
        return out, res
    return out
